# revision 25
# baseline (speedup 1.0000x reference)
"""BiLSTM-CRF Viterbi decode kernel for Trainium2 (Bass/Tile), 8-core SPMD.

Problem: feats (S=512, B=512, T=64) emissions, mask (B, S) contiguous-prefix,
transitions (T, T), start/end (T,). Output: decoded tag paths (B, S) int32.

Strategy
--------
Data-parallel over batch: 8 cores x 64 batches. Each core runs the Viterbi
forward scan (511 sequential steps) with the tag dim split in half across
partitions: state v[(ihi, b), i32] = v[b, ihi*32 + i32] on 128 partitions.

Per step, scores[(ihi,b), (jr, i32)] = fl(trans[i, j(jr)] + v[b, i]) where
jr is a *group-reordered* j index: each partition group's own 32 j's come
first (jr 0:32 == j owned by this group), the other group's second. The max
over i for each j then needs one partial reduce per half plus ONE cross-half
PE permutation matmul (bitwise exact - products are x*1.0):

  sc = trans_r + v         (adds split DVE/Pool in tuned chunks; the
                            gpsimd ucode only implements add/mult, so all
                            grouped max-reduces run on DVE)
  mh = max_i32 sc          (DVE grouped reduces, chunk-pipelined behind
                            the Pool adds)
  msw = P_swap @ mhF       (PE: partition-half swap into PSUM)
  best = max(mhO, msw); beste = best + e   (exact: max commutes with
                                            monotone fl-add)
  v' = m ? beste : v       (copy_predicated, in place)

All of feats lives in SBUF (one 8MB preload; 64KB/partition), so the
forward loop has zero input DMA; v is stored to DRAM once per step for
the backtrack.

Backpointers are NOT computed in the forward pass. The backtrack recomputes
the single needed argmax per (batch, step) from the stored v:

  T_lp[b,:] = trans[:, lp_b]  gathered bitwise-exactly via one-hot
                              PE transpose+matmul
  cand3     = fl(fl(v + T_lp) + emis[b, lp_b])  == reference rounding
  bp        = max_index(cand3)  (first-max tie-break == jnp.argmax)

v and emissions for the backtrack are prefetched in 8-step batched DMAs
(natural [b, t] layout), keeping all DMA off the serial critical path.

Mask/boundary effects (insert last_path at len-1, zeros beyond) are folded
algebraically: dec_i = bp*m_{i+1} + (i == len-1)*last_path. Lengths are
guaranteed >= S//2 (contiguous-prefix mask), so forward steps s < S//2 skip
the copy_predicated blend entirely (mask == 1 on every lane there).
"""
import os
import sys

sys.path.insert(0, "/opt/trn_rl_repo")

import numpy as np
from contextlib import ExitStack

import concourse.bass as bass
import concourse.tile as tile
from concourse import bacc, mybir
from concourse.bass_utils import run_bass_kernel_spmd

F32 = mybir.dt.float32
U32 = mybir.dt.uint32
A = mybir.AluOpType
AX = mybir.AxisListType

S, B, T = 512, 512, 64
NCORES = 8
BL = B // NCORES  # 64 batches per core
H = T // 2        # 32: tag half
P2 = 2 * BL       # 128 partitions
G = 32            # j-groups per half
DA = 12           # jr-groups per half added on DVE (rest: Pool)
W = 8             # backtrack prefetch chunk (steps per DMA)

_cached = {}


def build_program(n_steps=S):
    nc = bacc.Bacc("TRN2", target_bir_lowering=False, debug=False,
                   num_devices=NCORES)

    featsp = nc.dram_tensor("featsp", [P2, n_steps * H], F32, kind="ExternalInput").ap()
    featsn = nc.dram_tensor("featsn", [n_steps, BL, T], F32, kind="ExternalInput").ap()
    transbiR = nc.dram_tensor("transbiR", [P2, T * H], F32, kind="ExternalInput").ap()
    transTd = nc.dram_tensor("transTd", [T, T], F32, kind="ExternalInput").ap()
    startsp = nc.dram_tensor("startsp", [P2, H], F32, kind="ExternalInput").ap()
    swapd = nc.dram_tensor("swapd", [P2, P2], F32, kind="ExternalInput").ap()
    m2i_all = nc.dram_tensor("m2i_all", [P2, n_steps], mybir.dt.int32, kind="ExternalInput").ap()
    m_all = nc.dram_tensor("m_all", [BL, n_steps], F32, kind="ExternalInput").ap()
    onehL = nc.dram_tensor("onehL", [BL, n_steps], F32, kind="ExternalInput").ap()
    endb = nc.dram_tensor("endb", [BL, T], F32, kind="ExternalInput").ap()
    iotad = nc.dram_tensor("iotad", [BL, T], F32, kind="ExternalInput").ap()
    identd = nc.dram_tensor("identd", [T, T], F32, kind="ExternalInput").ap()

    vstore = nc.dram_tensor("vstore", [n_steps, P2, H], F32).ap()
    dec_out = nc.dram_tensor("dec_out", [BL, n_steps], F32, kind="ExternalOutput").ap()

    def r3(ap):
        return ap.rearrange("p (j i) -> p j i", i=H)

    with tile.TileContext(nc) as tc, ExitStack() as ctx:
        statics = ctx.enter_context(tc.tile_pool(name="statics", bufs=1))
        spool = ctx.enter_context(tc.tile_pool(name="spool", bufs=2))
        tpool = ctx.enter_context(tc.tile_pool(name="tpool", bufs=2))
        mpool = ctx.enter_context(tc.tile_pool(name="mpool", bufs=2))
        btpool = ctx.enter_context(tc.tile_pool(name="btpool", bufs=6))
        ringpool = ctx.enter_context(tc.tile_pool(name="ring", bufs=3))
        pspool = ctx.enter_context(tc.tile_pool(name="pspool", bufs=2, space="PSUM"))

        # ---- statics to SBUF ----
        # Forward-critical tensors first; feats split so the scan starts
        # after the first slice; backtrack-only statics load last (overlap
        # with the running forward).
        t_transbiR = statics.tile([P2, T * H], F32)
        nc.sync.dma_start(t_transbiR[:], transbiR)
        t_startsp = statics.tile([P2, H], F32)
        nc.sync.dma_start(t_startsp[:], startsp)
        t_swap = statics.tile([P2, P2], F32)
        nc.sync.dma_start(t_swap[:], swapd)
        t_m2i = statics.tile([P2, n_steps], mybir.dt.int32)
        nc.sync.dma_start(t_m2i[:], m2i_all)
        t_feats = statics.tile([P2, n_steps * H], F32)
        FCH = 32 * H  # 32-step feats slices
        nc.sync.dma_start(t_feats[:, 0:FCH], featsp[:, 0:FCH])
        for f0 in range(FCH, n_steps * H, 4 * FCH):
            f1 = min(f0 + 4 * FCH, n_steps * H)
            nc.sync.dma_start(t_feats[:, f0:f1], featsp[:, f0:f1])
        t_transT = statics.tile([T, T], F32)
        nc.sync.dma_start(t_transT[:], transTd)
        t_m = statics.tile([BL, n_steps], F32)
        nc.sync.dma_start(t_m[:], m_all)
        t_onehL = statics.tile([BL, n_steps], F32)
        nc.sync.dma_start(t_onehL[:], onehL)
        t_endb = statics.tile([BL, T], F32)
        nc.sync.dma_start(t_endb[:], endb)
        t_iota = statics.tile([BL, T], F32)
        nc.sync.dma_start(t_iota[:], iotad)
        t_ident = statics.tile([T, T], F32)
        nc.sync.dma_start(t_ident[:], identd)
        t_dec = statics.tile([BL, n_steps], F32)
        t_addend = statics.tile([BL, n_steps], F32)

        # ---- v0 (in-place state tile) ----
        v = statics.tile([P2, H], F32)
        nc.vector.tensor_add(v[:], t_startsp[:], t_feats[:, 0:H])
        nc.sync.dma_start(vstore[0], v[:])

        # ---- forward ----
        # Per step: scores chunks produced by DVE ("D") or Pool ("P") adds;
        # all grouped max-reduces on DVE. jr-group chunk plan, F-half
        # (jr 32:64, feeds the PE swap) first; trailing Pool chunks shrink so
        # the DVE tail isn't serialized behind a big Pool chunk.
        from concourse.tile_rust import add_dep_helper
        # All DVE-added groups live in the F-half so the D add+reduce are one
        # instruction each and mhF completes early (PE swap off critical path).
        PLAN = [("D", 32, 32 + 2 * DA),
                ("P", 32 + 2 * DA, 64),
                ("P", 0, 13), ("P", 13, 26), ("P", 26, 32)]
        for s in range(1, n_steps):
            e_s = t_feats[:, s * H:(s + 1) * H]
            vb = v[:, None, :]
            mhF = mpool.tile([P2, G], F32, tag="mhF")
            mhO = mpool.tile([P2, G], F32, tag="mhO")

            # adds first (DVE chunks immediately; Pool chunks in queue order)
            chunks = []
            prevp = None
            for ci, (eng, j0, j1) in enumerate(PLAN):
                sc = spool.tile([P2, (j1 - j0) * H], F32, tag=f"sc{ci}")
                scv = sc[:].rearrange("p (j i) -> p j i", i=H)
                args = (scv, r3(t_transbiR[:])[:, j0:j1, :],
                        vb.to_broadcast([P2, j1 - j0, H]))
                if eng == "D":
                    nc.vector.tensor_add(*args)
                else:
                    p = nc.gpsimd.tensor_add(*args)
                    if prevp is not None:
                        add_dep_helper(p.ins, prevp.ins, sync=False,
                                       reason="pool chunk order")
                    prevp = p
                chunks.append((scv, j0, j1))

            # grouped reduces on DVE, in plan order
            prevr = None
            for ci, (scv, j0, j1) in enumerate(chunks):
                if j0 >= G:
                    dst = mhF[:, j0 - G:j1 - G]
                else:
                    dst = mhO[:, j0:j1]
                r = nc.vector.tensor_reduce(dst, scv, axis=AX.X, op=A.max)
                if prevr is not None:
                    add_dep_helper(r.ins, prevr.ins, sync=False,
                                   reason="reduce order on DVE")
                prevr = r
                if j1 == 2 * G:  # mhF complete -> cross-half swap on PE
                    msw = pspool.tile([P2, G], F32, tag="msw")
                    nc.tensor.matmul(msw[:], t_swap[:], mhF[:],
                                     start=True, stop=True)

            if s < S // 2:
                # lengths are >= S//2, so mask == 1 on every lane here: the
                # blend is an unconditional write and the e-add can target
                # the state tile directly (same WAR pattern as copy_pred)
                best = mpool.tile([P2, G], F32, tag="beste")
                nc.vector.tensor_tensor(best[:], mhO[:], msw[:], op=A.max)
                nc.vector.tensor_add(v[:], best[:], e_s)
            else:
                beste = mpool.tile([P2, G], F32, tag="beste")
                nc.vector.tensor_tensor(beste[:], mhO[:], msw[:], op=A.max)
                nc.vector.tensor_add(beste[:], beste[:], e_s)
                nc.vector.copy_predicated(v[:],
                                          t_m2i[:, s:s + 1].to_broadcast([P2, H]),
                                          beste[:])
            nc.sync.dma_start(vstore[s], v[:])

        # ---- epilogue: last_path ----
        vnat = statics.tile([BL, T], F32)
        nc.vector.tensor_copy(vnat[:, 0:H], v[0:BL, :])
        nc.sync.dma_start(vnat[:, H:T], v[BL:P2, :])

        fv = statics.tile([BL, T], F32)
        nc.vector.tensor_add(fv[:], vnat[:], t_endb[:])
        fv8 = statics.tile([BL, 8], F32)
        nc.vector.max(out=fv8[:], in_=fv[:])
        fvi = statics.tile([BL, 8], U32)
        nc.vector.max_index(fvi[:], fv8[:], fv[:])
        nc.vector.tensor_copy(t_dec[:, n_steps - 1:n_steps], fvi[:, 0:1])
        nc.vector.tensor_scalar(t_addend[:], t_onehL[:],
                                t_dec[:, n_steps - 1:n_steps], None, op0=A.mult)

        # ---- backtrack: batched v/e prefetch, per-step argmax recompute ----
        # chunk c covers steps [c*W, c*W+W); processed descending
        nch = (n_steps - 1 + W - 1) // W

        vst4 = vstore.rearrange("s (h b) i -> s h b i", h=2)

        def prefetch(c):
            c0 = c * W
            cw = min(W, (n_steps - 1) - c0)  # steps c0..c0+cw-1
            if cw <= 0:
                return None, None, 0
            vt = ringpool.tile([BL, W * T], F32, tag="vt")
            src = vst4[c0:c0 + cw].rearrange("s h b i -> b s h i")
            nc.scalar.dma_start(
                vt[:].rearrange("b (s h i) -> b s h i", h=2, i=H)[:, 0:cw], src)
            et = ringpool.tile([BL, W * T], F32, tag="et")
            esrc = featsn[c0 + 1:c0 + cw + 1].rearrange("s b t -> b s t")
            nc.scalar.dma_start(
                et[:].rearrange("b (s t) -> b s t", t=T)[:, 0:cw, :], esrc)
            return vt, et, cw

        bufs = {}
        for c in range(nch - 1, max(nch - 3, -1), -1):
            bufs[c] = prefetch(c)

        for i in range(n_steps - 2, -1, -1):
            c = i // W
            w = i - c * W
            if c - 2 >= 0 and c - 2 not in bufs:
                bufs[c - 2] = prefetch(c - 2)
            vt_t, et_t, cw = bufs[c]
            vt = vt_t[:, w * T:(w + 1) * T]
            et = et_t[:, w * T:(w + 1) * T]

            lp_ap = t_dec[:, i + 1:i + 2]
            onehot = btpool.tile([BL, T], F32, tag="onehot")
            nc.vector.tensor_scalar(onehot[:], t_iota[:], lp_ap, None,
                                    op0=A.is_equal)

            prod = btpool.tile([BL, T], F32, tag="prod")
            nc.vector.tensor_mul(prod[:], et, onehot[:])
            elp = btpool.tile([BL, 1], F32, tag="elp")
            nc.vector.tensor_reduce(elp[:], prod[:], axis=AX.X, op=A.add)

            p_ohT = pspool.tile([T, BL], F32, tag="p_ohT")
            nc.tensor.transpose(p_ohT[:], onehot[:], t_ident[:])
            ohT = btpool.tile([T, BL], F32, tag="ohT")
            nc.vector.tensor_copy(ohT[:], p_ohT[:])
            p_tlp = pspool.tile([BL, T], F32, tag="p_tlp")
            # psum = I @ vt first (no ohT dependency - fires early), then
            # accumulate trans[:, lp].T: fl(v + T_lp), exact (2 addends)
            nc.tensor.matmul(p_tlp[:], t_ident[:], vt, start=True, stop=False)
            nc.tensor.matmul(p_tlp[:], ohT[:], t_transT[:], start=False, stop=True)

            cand3 = btpool.tile([BL, T], F32, tag="cand3")
            nc.vector.tensor_scalar(cand3[:], p_tlp[:], elp[:, 0:1], None,
                                    op0=A.add)

            c8 = btpool.tile([BL, 8], F32, tag="c8")
            nc.vector.max(out=c8[:], in_=cand3[:])
            ci = btpool.tile([BL, 8], U32, tag="ci")
            nc.vector.max_index(ci[:], c8[:], cand3[:])

            nc.vector.tensor_scalar(t_dec[:, i:i + 1], ci[:, 0:1],
                                    t_m[:, i + 1:i + 2], t_addend[:, i:i + 1],
                                    op0=A.mult, op1=A.add)

        nc.sync.dma_start(dec_out, t_dec[:])

    nc.compile()
    return nc


def host_prep(feats, mask, start_transitions, end_transitions, transitions,
              n_steps=S):
    feats = np.asarray(feats, dtype=np.float32)
    mask = np.asarray(mask, dtype=np.float32)
    start = np.asarray(start_transitions, dtype=np.float32)
    end = np.asarray(end_transitions, dtype=np.float32)
    trans = np.asarray(transitions, dtype=np.float32)

    # transbiR[(ihi*BL+b), jr*H+i32] = trans[ihi*H+i32, jmap(ihi, jr)]
    # jmap(0, jr) = jr; jmap(1, jr) = (jr + H) % T  (own j's first per group)
    transbiR = np.empty((P2, T * H), dtype=np.float32)
    blk0 = np.ascontiguousarray(trans[0:H, :].T)           # [T(j), H(i32)]
    transbiR[0:BL, :] = np.tile(blk0.reshape(1, T * H), (BL, 1))
    blk1 = np.ascontiguousarray(trans[H:T, :].T)           # [T(j), H(i32)]
    blk1r = np.concatenate([blk1[H:], blk1[:H]], axis=0)   # j = (jr+H)%T
    transbiR[BL:P2, :] = np.tile(blk1r.reshape(1, T * H), (BL, 1))

    transT = np.ascontiguousarray(trans.T)
    startsp = np.empty((P2, H), dtype=np.float32)
    for ihi in range(2):
        startsp[ihi * BL:(ihi + 1) * BL, :] = np.tile(
            start[ihi * H:(ihi + 1) * H].reshape(1, H), (BL, 1))
    swapd = np.roll(np.eye(P2, dtype=np.float32), BL, axis=0)
    endb = np.tile(end.reshape(1, T), (BL, 1))
    iotad = np.tile(np.arange(T, dtype=np.float32).reshape(1, T), (BL, 1))
    identd = np.eye(T, dtype=np.float32)

    lengths = mask.sum(axis=1).astype(np.int64)

    in_maps = []
    for c in range(NCORES):
        b0 = c * BL
        fc = feats[:n_steps, b0:b0 + BL, :]                      # [S, BL, T]
        featsp = np.ascontiguousarray(
            fc.reshape(n_steps, BL, 2, H).transpose(2, 1, 0, 3)
        ).reshape(P2, n_steps * H)
        msk = np.ascontiguousarray(mask[b0:b0 + BL, :n_steps])
        msk2 = np.concatenate([msk, msk], axis=0)
        onehL = (np.arange(n_steps)[None, :] == (lengths[b0:b0 + BL, None] - 1))
        in_maps.append(dict(
            featsp=featsp,
            featsn=np.ascontiguousarray(fc),
            transbiR=transbiR, transTd=transT, startsp=startsp, swapd=swapd,
            m2i_all=msk2.astype(np.int32),
            m_all=msk, onehL=onehL.astype(np.float32),
            endb=endb, iotad=iotad, identd=identd,
        ))
    return in_maps


def kernel(feats, mask, start_transitions, end_transitions, transitions):
    if "nc" not in _cached:
        _cached["nc"] = build_program(S)
    nc = _cached["nc"]
    in_maps = host_prep(feats, mask, start_transitions, end_transitions,
                        transitions, S)
    res = run_bass_kernel_spmd(nc, in_maps, list(range(NCORES)))
    out = np.empty((B, S), dtype=np.int32)
    for c in range(NCORES):
        out[c * BL:(c + 1) * BL, :] = np.rint(
            res.results[c]["dec_out"]).astype(np.int32)
    return out


# revision 26
# speedup vs baseline: 1.0002x; 1.0002x over previous
"""BiLSTM-CRF Viterbi decode kernel for Trainium2 (Bass/Tile), 8-core SPMD.

Problem: feats (S=512, B=512, T=64) emissions, mask (B, S) contiguous-prefix,
transitions (T, T), start/end (T,). Output: decoded tag paths (B, S) int32.

Strategy
--------
Data-parallel over batch: 8 cores x 64 batches. Each core runs the Viterbi
forward scan (511 sequential steps) with the tag dim split in half across
partitions: state v[(ihi, b), i32] = v[b, ihi*32 + i32] on 128 partitions.

Per step, scores[(ihi,b), (jr, i32)] = fl(trans[i, j(jr)] + v[b, i]) where
jr is a *group-reordered* j index: each partition group's own 32 j's come
first (jr 0:32 == j owned by this group), the other group's second. The max
over i for each j then needs one partial reduce per half plus ONE cross-half
PE permutation matmul (bitwise exact - products are x*1.0):

  sc = trans_r + v         (adds split DVE/Pool in tuned chunks; the
                            gpsimd ucode only implements add/mult, so all
                            grouped max-reduces run on DVE)
  mh = max_i32 sc          (DVE grouped reduces, chunk-pipelined behind
                            the Pool adds)
  msw = P_swap @ mhF       (PE: partition-half swap into PSUM)
  best = max(mhO, msw); beste = best + e   (exact: max commutes with
                                            monotone fl-add)
  v' = m ? beste : v       (copy_predicated, in place)

All of feats lives in SBUF (one 8MB preload; 64KB/partition), so the
forward loop has zero input DMA; v is stored to DRAM once per step for
the backtrack.

Backpointers are NOT computed in the forward pass. The backtrack recomputes
the single needed argmax per (batch, step) from the stored v:

  T_lp[b,:] = trans[:, lp_b]  gathered bitwise-exactly via one-hot
                              PE transpose+matmul
  cand3     = fl(fl(v + T_lp) + emis[b, lp_b])  == reference rounding
  bp        = max_index(cand3)  (first-max tie-break == jnp.argmax)

v and emissions for the backtrack are prefetched in 8-step batched DMAs
(natural [b, t] layout), keeping all DMA off the serial critical path.

Mask/boundary effects (insert last_path at len-1, zeros beyond) are folded
algebraically: dec_i = bp*m_{i+1} + (i == len-1)*last_path. Lengths are
guaranteed >= S//2 (contiguous-prefix mask), so forward steps s < S//2 skip
the copy_predicated blend entirely (mask == 1 on every lane there).
"""
import os
import sys

sys.path.insert(0, "/opt/trn_rl_repo")

import numpy as np
from contextlib import ExitStack

import concourse.bass as bass
import concourse.tile as tile
from concourse import bacc, mybir
from concourse.bass_utils import run_bass_kernel_spmd

F32 = mybir.dt.float32
U32 = mybir.dt.uint32
A = mybir.AluOpType
AX = mybir.AxisListType

S, B, T = 512, 512, 64
NCORES = 8
BL = B // NCORES  # 64 batches per core
H = T // 2        # 32: tag half
P2 = 2 * BL       # 128 partitions
G = 32            # j-groups per half
DA = 12           # jr-groups per half added on DVE (rest: Pool)
W = 8             # backtrack prefetch chunk (steps per DMA)

_cached = {}


def build_program(n_steps=S):
    nc = bacc.Bacc("TRN2", target_bir_lowering=False, debug=False,
                   num_devices=NCORES)

    featsp = nc.dram_tensor("featsp", [P2, n_steps * H], F32, kind="ExternalInput").ap()
    featsn = nc.dram_tensor("featsn", [n_steps, BL, T], F32, kind="ExternalInput").ap()
    transbiR = nc.dram_tensor("transbiR", [P2, T * H], F32, kind="ExternalInput").ap()
    transTd = nc.dram_tensor("transTd", [T, T], F32, kind="ExternalInput").ap()
    startsp = nc.dram_tensor("startsp", [P2, H], F32, kind="ExternalInput").ap()
    swapd = nc.dram_tensor("swapd", [P2, P2], F32, kind="ExternalInput").ap()
    m2i_all = nc.dram_tensor("m2i_all", [P2, n_steps], mybir.dt.int32, kind="ExternalInput").ap()
    m_all = nc.dram_tensor("m_all", [BL, n_steps], F32, kind="ExternalInput").ap()
    onehL = nc.dram_tensor("onehL", [BL, n_steps], F32, kind="ExternalInput").ap()
    endb = nc.dram_tensor("endb", [BL, T], F32, kind="ExternalInput").ap()
    iotad = nc.dram_tensor("iotad", [BL, T], F32, kind="ExternalInput").ap()
    identd = nc.dram_tensor("identd", [T, T], F32, kind="ExternalInput").ap()

    vstore = nc.dram_tensor("vstore", [n_steps, P2, H], F32).ap()
    dec_out = nc.dram_tensor("dec_out", [BL, n_steps], F32, kind="ExternalOutput").ap()

    def r3(ap):
        return ap.rearrange("p (j i) -> p j i", i=H)

    with tile.TileContext(nc) as tc, ExitStack() as ctx:
        statics = ctx.enter_context(tc.tile_pool(name="statics", bufs=1))
        spool = ctx.enter_context(tc.tile_pool(name="spool", bufs=2))
        tpool = ctx.enter_context(tc.tile_pool(name="tpool", bufs=2))
        mpool = ctx.enter_context(tc.tile_pool(name="mpool", bufs=2))
        btpool = ctx.enter_context(tc.tile_pool(name="btpool", bufs=6))
        ringpool = ctx.enter_context(tc.tile_pool(name="ring", bufs=3))
        pspool = ctx.enter_context(tc.tile_pool(name="pspool", bufs=2, space="PSUM"))

        # ---- statics to SBUF ----
        # Forward-critical tensors first; feats split so the scan starts
        # after the first slice; backtrack-only statics load last (overlap
        # with the running forward).
        t_transbiR = statics.tile([P2, T * H], F32)
        nc.sync.dma_start(t_transbiR[:], transbiR)
        t_startsp = statics.tile([P2, H], F32)
        nc.sync.dma_start(t_startsp[:], startsp)
        t_swap = statics.tile([P2, P2], F32)
        nc.sync.dma_start(t_swap[:], swapd)
        t_m2i = statics.tile([P2, n_steps], mybir.dt.int32)
        nc.sync.dma_start(t_m2i[:], m2i_all)
        t_feats = statics.tile([P2, n_steps * H], F32)
        FCH = 32 * H  # 32-step feats slices
        nc.sync.dma_start(t_feats[:, 0:FCH], featsp[:, 0:FCH])
        for f0 in range(FCH, n_steps * H, 4 * FCH):
            f1 = min(f0 + 4 * FCH, n_steps * H)
            nc.sync.dma_start(t_feats[:, f0:f1], featsp[:, f0:f1])
        t_transT = statics.tile([T, T], F32)
        nc.sync.dma_start(t_transT[:], transTd)
        t_m = statics.tile([BL, n_steps], F32)
        nc.sync.dma_start(t_m[:], m_all)
        t_onehL = statics.tile([BL, n_steps], F32)
        nc.sync.dma_start(t_onehL[:], onehL)
        t_endb = statics.tile([BL, T], F32)
        nc.sync.dma_start(t_endb[:], endb)
        t_iota = statics.tile([BL, T], F32)
        nc.sync.dma_start(t_iota[:], iotad)
        t_ident = statics.tile([T, T], F32)
        nc.sync.dma_start(t_ident[:], identd)
        t_dec = statics.tile([BL, n_steps], F32)
        t_addend = statics.tile([BL, n_steps], F32)

        # ---- v0 (in-place state tile) ----
        v = statics.tile([P2, H], F32)
        nc.vector.tensor_add(v[:], t_startsp[:], t_feats[:, 0:H])
        nc.sync.dma_start(vstore[0], v[:])

        # ---- forward ----
        # Per step: scores chunks produced by DVE ("D") or Pool ("P") adds;
        # all grouped max-reduces on DVE. jr-group chunk plan, F-half
        # (jr 32:64, feeds the PE swap) first; trailing Pool chunks shrink so
        # the DVE tail isn't serialized behind a big Pool chunk.
        from concourse.tile_rust import add_dep_helper
        # All DVE-added groups live in the F-half so the D add+reduce are one
        # instruction each and mhF completes early (PE swap off critical path).
        PLAN = [("D", 32, 32 + 2 * DA),
                ("P", 32 + 2 * DA, 64),
                ("P", 0, 13), ("P", 13, 26), ("P", 26, 32)]
        for s in range(1, n_steps):
            e_s = t_feats[:, s * H:(s + 1) * H]
            vb = v[:, None, :]
            mhF = mpool.tile([P2, G], F32, tag="mhF")
            mhO = mpool.tile([P2, G], F32, tag="mhO")

            # adds first (DVE chunks immediately; Pool chunks in queue order)
            chunks = []
            prevp = None
            for ci, (eng, j0, j1) in enumerate(PLAN):
                sc = spool.tile([P2, (j1 - j0) * H], F32, tag=f"sc{ci}")
                scv = sc[:].rearrange("p (j i) -> p j i", i=H)
                args = (scv, r3(t_transbiR[:])[:, j0:j1, :],
                        vb.to_broadcast([P2, j1 - j0, H]))
                if eng == "D":
                    nc.vector.tensor_add(*args)
                else:
                    p = nc.gpsimd.tensor_add(*args)
                    if prevp is not None:
                        add_dep_helper(p.ins, prevp.ins, sync=False,
                                       reason="pool chunk order")
                    prevp = p
                chunks.append((scv, j0, j1))

            # grouped reduces on DVE, in plan order
            prevr = None
            for ci, (scv, j0, j1) in enumerate(chunks):
                if j0 >= G:
                    dst = mhF[:, j0 - G:j1 - G]
                else:
                    dst = mhO[:, j0:j1]
                r = nc.vector.tensor_reduce(dst, scv, axis=AX.X, op=A.max)
                if prevr is not None:
                    add_dep_helper(r.ins, prevr.ins, sync=False,
                                   reason="reduce order on DVE")
                prevr = r
                if j1 == 2 * G:  # mhF complete -> cross-half swap on PE
                    msw = pspool.tile([P2, G], F32, tag="msw")
                    nc.tensor.matmul(msw[:], t_swap[:], mhF[:],
                                     start=True, stop=True)

            if s < S // 2:
                # lengths are >= S//2, so mask == 1 on every lane here: the
                # blend is an unconditional write and the e-add can target
                # the state tile directly (same WAR pattern as copy_pred)
                best = mpool.tile([P2, G], F32, tag="beste")
                nc.vector.tensor_tensor(best[:], mhO[:], msw[:], op=A.max)
                nc.vector.tensor_add(v[:], best[:], e_s)
            else:
                beste = mpool.tile([P2, G], F32, tag="beste")
                nc.vector.tensor_tensor(beste[:], mhO[:], msw[:], op=A.max)
                nc.vector.tensor_add(beste[:], beste[:], e_s)
                nc.vector.copy_predicated(v[:],
                                          t_m2i[:, s:s + 1].to_broadcast([P2, H]),
                                          beste[:])
            if s < n_steps - 1:  # vstore[n_steps-1] is never read back
                nc.sync.dma_start(vstore[s], v[:])

        # ---- epilogue: last_path ----
        vnat = statics.tile([BL, T], F32)
        nc.vector.tensor_copy(vnat[:, 0:H], v[0:BL, :])
        nc.sync.dma_start(vnat[:, H:T], v[BL:P2, :])

        fv = statics.tile([BL, T], F32)
        nc.vector.tensor_add(fv[:], vnat[:], t_endb[:])
        fv8 = statics.tile([BL, 8], F32)
        nc.vector.max(out=fv8[:], in_=fv[:])
        fvi = statics.tile([BL, 8], U32)
        nc.vector.max_index(fvi[:], fv8[:], fv[:])
        nc.vector.tensor_copy(t_dec[:, n_steps - 1:n_steps], fvi[:, 0:1])
        nc.vector.tensor_scalar(t_addend[:], t_onehL[:],
                                t_dec[:, n_steps - 1:n_steps], None, op0=A.mult)

        # ---- backtrack: batched v/e prefetch, per-step argmax recompute ----
        # chunk c covers steps [c*W, c*W+W); processed descending
        nch = (n_steps - 1 + W - 1) // W

        vst4 = vstore.rearrange("s (h b) i -> s h b i", h=2)

        def prefetch(c):
            c0 = c * W
            cw = min(W, (n_steps - 1) - c0)  # steps c0..c0+cw-1
            if cw <= 0:
                return None, None, 0
            vt = ringpool.tile([BL, W * T], F32, tag="vt")
            src = vst4[c0:c0 + cw].rearrange("s h b i -> b s h i")
            nc.scalar.dma_start(
                vt[:].rearrange("b (s h i) -> b s h i", h=2, i=H)[:, 0:cw], src)
            et = ringpool.tile([BL, W * T], F32, tag="et")
            esrc = featsn[c0 + 1:c0 + cw + 1].rearrange("s b t -> b s t")
            nc.scalar.dma_start(
                et[:].rearrange("b (s t) -> b s t", t=T)[:, 0:cw, :], esrc)
            return vt, et, cw

        bufs = {}
        for c in range(nch - 1, max(nch - 3, -1), -1):
            bufs[c] = prefetch(c)

        for i in range(n_steps - 2, -1, -1):
            c = i // W
            w = i - c * W
            if c - 2 >= 0 and c - 2 not in bufs:
                bufs[c - 2] = prefetch(c - 2)
            vt_t, et_t, cw = bufs[c]
            vt = vt_t[:, w * T:(w + 1) * T]
            et = et_t[:, w * T:(w + 1) * T]

            lp_ap = t_dec[:, i + 1:i + 2]
            onehot = btpool.tile([BL, T], F32, tag="onehot")
            nc.vector.tensor_scalar(onehot[:], t_iota[:], lp_ap, None,
                                    op0=A.is_equal)

            prod = btpool.tile([BL, T], F32, tag="prod")
            nc.vector.tensor_mul(prod[:], et, onehot[:])
            elp = btpool.tile([BL, 1], F32, tag="elp")
            nc.vector.tensor_reduce(elp[:], prod[:], axis=AX.X, op=A.add)

            p_ohT = pspool.tile([T, BL], F32, tag="p_ohT")
            nc.tensor.transpose(p_ohT[:], onehot[:], t_ident[:])
            ohT = btpool.tile([T, BL], F32, tag="ohT")
            nc.vector.tensor_copy(ohT[:], p_ohT[:])
            p_tlp = pspool.tile([BL, T], F32, tag="p_tlp")
            # psum = I @ vt first (no ohT dependency - fires early), then
            # accumulate trans[:, lp].T: fl(v + T_lp), exact (2 addends)
            nc.tensor.matmul(p_tlp[:], t_ident[:], vt, start=True, stop=False)
            nc.tensor.matmul(p_tlp[:], ohT[:], t_transT[:], start=False, stop=True)

            cand3 = btpool.tile([BL, T], F32, tag="cand3")
            nc.vector.tensor_scalar(cand3[:], p_tlp[:], elp[:, 0:1], None,
                                    op0=A.add)

            c8 = btpool.tile([BL, 8], F32, tag="c8")
            nc.vector.max(out=c8[:], in_=cand3[:])
            ci = btpool.tile([BL, 8], U32, tag="ci")
            nc.vector.max_index(ci[:], c8[:], cand3[:])

            nc.vector.tensor_scalar(t_dec[:, i:i + 1], ci[:, 0:1],
                                    t_m[:, i + 1:i + 2], t_addend[:, i:i + 1],
                                    op0=A.mult, op1=A.add)

        nc.sync.dma_start(dec_out, t_dec[:])

    nc.compile()
    return nc


def host_prep(feats, mask, start_transitions, end_transitions, transitions,
              n_steps=S):
    feats = np.asarray(feats, dtype=np.float32)
    mask = np.asarray(mask, dtype=np.float32)
    start = np.asarray(start_transitions, dtype=np.float32)
    end = np.asarray(end_transitions, dtype=np.float32)
    trans = np.asarray(transitions, dtype=np.float32)

    # transbiR[(ihi*BL+b), jr*H+i32] = trans[ihi*H+i32, jmap(ihi, jr)]
    # jmap(0, jr) = jr; jmap(1, jr) = (jr + H) % T  (own j's first per group)
    transbiR = np.empty((P2, T * H), dtype=np.float32)
    blk0 = np.ascontiguousarray(trans[0:H, :].T)           # [T(j), H(i32)]
    transbiR[0:BL, :] = np.tile(blk0.reshape(1, T * H), (BL, 1))
    blk1 = np.ascontiguousarray(trans[H:T, :].T)           # [T(j), H(i32)]
    blk1r = np.concatenate([blk1[H:], blk1[:H]], axis=0)   # j = (jr+H)%T
    transbiR[BL:P2, :] = np.tile(blk1r.reshape(1, T * H), (BL, 1))

    transT = np.ascontiguousarray(trans.T)
    startsp = np.empty((P2, H), dtype=np.float32)
    for ihi in range(2):
        startsp[ihi * BL:(ihi + 1) * BL, :] = np.tile(
            start[ihi * H:(ihi + 1) * H].reshape(1, H), (BL, 1))
    swapd = np.roll(np.eye(P2, dtype=np.float32), BL, axis=0)
    endb = np.tile(end.reshape(1, T), (BL, 1))
    iotad = np.tile(np.arange(T, dtype=np.float32).reshape(1, T), (BL, 1))
    identd = np.eye(T, dtype=np.float32)

    lengths = mask.sum(axis=1).astype(np.int64)

    in_maps = []
    for c in range(NCORES):
        b0 = c * BL
        fc = feats[:n_steps, b0:b0 + BL, :]                      # [S, BL, T]
        featsp = np.ascontiguousarray(
            fc.reshape(n_steps, BL, 2, H).transpose(2, 1, 0, 3)
        ).reshape(P2, n_steps * H)
        msk = np.ascontiguousarray(mask[b0:b0 + BL, :n_steps])
        msk2 = np.concatenate([msk, msk], axis=0)
        onehL = (np.arange(n_steps)[None, :] == (lengths[b0:b0 + BL, None] - 1))
        in_maps.append(dict(
            featsp=featsp,
            featsn=np.ascontiguousarray(fc),
            transbiR=transbiR, transTd=transT, startsp=startsp, swapd=swapd,
            m2i_all=msk2.astype(np.int32),
            m_all=msk, onehL=onehL.astype(np.float32),
            endb=endb, iotad=iotad, identd=identd,
        ))
    return in_maps


def kernel(feats, mask, start_transitions, end_transitions, transitions):
    if "nc" not in _cached:
        _cached["nc"] = build_program(S)
    nc = _cached["nc"]
    in_maps = host_prep(feats, mask, start_transitions, end_transitions,
                        transitions, S)
    res = run_bass_kernel_spmd(nc, in_maps, list(range(NCORES)))
    out = np.empty((B, S), dtype=np.int32)
    for c in range(NCORES):
        out[c * BL:(c + 1) * BL, :] = np.rint(
            res.results[c]["dec_out"]).astype(np.int32)
    return out


# revision 27
# speedup vs baseline: 1.0004x; 1.0002x over previous
"""BiLSTM-CRF Viterbi decode kernel for Trainium2 (Bass/Tile), 8-core SPMD.

Problem: feats (S=512, B=512, T=64) emissions, mask (B, S) contiguous-prefix,
transitions (T, T), start/end (T,). Output: decoded tag paths (B, S) int32.

Strategy
--------
Data-parallel over batch: 8 cores x 64 batches. Each core runs the Viterbi
forward scan (511 sequential steps) with the tag dim split in half across
partitions: state v[(ihi, b), i32] = v[b, ihi*32 + i32] on 128 partitions.

Per step, scores[(ihi,b), (jr, i32)] = fl(trans[i, j(jr)] + v[b, i]) where
jr is a *group-reordered* j index: each partition group's own 32 j's come
first (jr 0:32 == j owned by this group), the other group's second. The max
over i for each j then needs one partial reduce per half plus ONE cross-half
PE permutation matmul (bitwise exact - products are x*1.0):

  sc = trans_r + v         (adds split DVE/Pool in tuned chunks; the
                            gpsimd ucode only implements add/mult, so all
                            grouped max-reduces run on DVE)
  mh = max_i32 sc          (DVE grouped reduces, chunk-pipelined behind
                            the Pool adds)
  msw = P_swap @ mhF       (PE: partition-half swap into PSUM)
  best = max(mhO, msw); beste = best + e   (exact: max commutes with
                                            monotone fl-add)
  v' = m ? beste : v       (copy_predicated, in place)

All of feats lives in SBUF (one 8MB preload; 64KB/partition), so the
forward loop has zero input DMA; v is stored to DRAM once per step for
the backtrack.

Backpointers are NOT computed in the forward pass. The backtrack recomputes
the single needed argmax per (batch, step) from the stored v:

  T_lp[b,:] = trans[:, lp_b]  gathered bitwise-exactly via one-hot
                              PE transpose+matmul
  cand3     = fl(fl(v + T_lp) + emis[b, lp_b])  == reference rounding
  bp        = max_index(cand3)  (first-max tie-break == jnp.argmax)

v and emissions for the backtrack are prefetched in 8-step batched DMAs
(natural [b, t] layout), keeping all DMA off the serial critical path.

Mask/boundary effects (insert last_path at len-1, zeros beyond) are folded
algebraically: dec_i = bp*m_{i+1} + (i == len-1)*last_path. Lengths are
guaranteed >= S//2 (contiguous-prefix mask), so forward steps s < S//2 skip
the copy_predicated blend entirely (mask == 1 on every lane there).
"""
import os
import sys

sys.path.insert(0, "/opt/trn_rl_repo")

import numpy as np
from contextlib import ExitStack

import concourse.bass as bass
import concourse.tile as tile
from concourse import bacc, mybir
from concourse.bass_utils import run_bass_kernel_spmd

F32 = mybir.dt.float32
U32 = mybir.dt.uint32
A = mybir.AluOpType
AX = mybir.AxisListType

S, B, T = 512, 512, 64
NCORES = 8
BL = B // NCORES  # 64 batches per core
H = T // 2        # 32: tag half
P2 = 2 * BL       # 128 partitions
G = 32            # j-groups per half
DA = 12           # jr-groups per half added on DVE (rest: Pool)
W = 8             # backtrack prefetch chunk (steps per DMA)

_cached = {}


def build_program(n_steps=S):
    nc = bacc.Bacc("TRN2", target_bir_lowering=False, debug=False,
                   num_devices=NCORES)

    featsp = nc.dram_tensor("featsp", [P2, n_steps * H], F32, kind="ExternalInput").ap()
    featsn = nc.dram_tensor("featsn", [n_steps, BL, T], F32, kind="ExternalInput").ap()
    transbiR = nc.dram_tensor("transbiR", [P2, T * H], F32, kind="ExternalInput").ap()
    transTd = nc.dram_tensor("transTd", [T, T], F32, kind="ExternalInput").ap()
    startsp = nc.dram_tensor("startsp", [P2, H], F32, kind="ExternalInput").ap()
    swapd = nc.dram_tensor("swapd", [P2, P2], F32, kind="ExternalInput").ap()
    m2i_all = nc.dram_tensor("m2i_all", [P2, n_steps], mybir.dt.int32, kind="ExternalInput").ap()
    m_all = nc.dram_tensor("m_all", [BL, n_steps], F32, kind="ExternalInput").ap()
    onehL = nc.dram_tensor("onehL", [BL, n_steps], F32, kind="ExternalInput").ap()
    endb = nc.dram_tensor("endb", [BL, T], F32, kind="ExternalInput").ap()
    iotad = nc.dram_tensor("iotad", [BL, T], F32, kind="ExternalInput").ap()
    identd = nc.dram_tensor("identd", [T, T], F32, kind="ExternalInput").ap()

    vstore = nc.dram_tensor("vstore", [n_steps, P2, H], F32).ap()
    dec_out = nc.dram_tensor("dec_out", [BL, n_steps], F32, kind="ExternalOutput").ap()

    def r3(ap):
        return ap.rearrange("p (j i) -> p j i", i=H)

    with tile.TileContext(nc) as tc, ExitStack() as ctx:
        statics = ctx.enter_context(tc.tile_pool(name="statics", bufs=1))
        spool = ctx.enter_context(tc.tile_pool(name="spool", bufs=2))
        tpool = ctx.enter_context(tc.tile_pool(name="tpool", bufs=2))
        mpool = ctx.enter_context(tc.tile_pool(name="mpool", bufs=2))
        btpool = ctx.enter_context(tc.tile_pool(name="btpool", bufs=6))
        ringpool = ctx.enter_context(tc.tile_pool(name="ring", bufs=3))
        pspool = ctx.enter_context(tc.tile_pool(name="pspool", bufs=2, space="PSUM"))

        # ---- statics to SBUF ----
        # Forward-critical tensors first; feats split so the scan starts
        # after the first slice; backtrack-only statics load last (overlap
        # with the running forward).
        t_transbiR = statics.tile([P2, T * H], F32)
        nc.sync.dma_start(t_transbiR[:], transbiR)
        t_startsp = statics.tile([P2, H], F32)
        nc.sync.dma_start(t_startsp[:], startsp)
        t_swap = statics.tile([P2, P2], F32)
        nc.sync.dma_start(t_swap[:], swapd)
        t_m2i = statics.tile([P2, n_steps], mybir.dt.int32)
        nc.sync.dma_start(t_m2i[:], m2i_all)
        t_feats = statics.tile([P2, n_steps * H], F32)
        FCH = 32 * H  # 32-step feats slices
        nc.sync.dma_start(t_feats[:, 0:FCH], featsp[:, 0:FCH])
        for f0 in range(FCH, n_steps * H, 4 * FCH):
            f1 = min(f0 + 4 * FCH, n_steps * H)
            nc.sync.dma_start(t_feats[:, f0:f1], featsp[:, f0:f1])
        t_transT = statics.tile([T, T], F32)
        nc.sync.dma_start(t_transT[:], transTd)
        t_m = statics.tile([BL, n_steps], F32)
        nc.sync.dma_start(t_m[:], m_all)
        t_onehL = statics.tile([BL, n_steps], F32)
        nc.sync.dma_start(t_onehL[:], onehL)
        t_endb = statics.tile([BL, T], F32)
        nc.sync.dma_start(t_endb[:], endb)
        t_iota = statics.tile([BL, T], F32)
        nc.sync.dma_start(t_iota[:], iotad)
        t_ident = statics.tile([T, T], F32)
        nc.sync.dma_start(t_ident[:], identd)
        t_dec = statics.tile([BL, n_steps], F32)
        t_addend = statics.tile([BL, n_steps], F32)

        # ---- v0 (in-place state tile) ----
        v = statics.tile([P2, H], F32)
        nc.vector.tensor_add(v[:], t_startsp[:], t_feats[:, 0:H])
        # scalar queue: the sync queue is ~25us deep in statics DMAs here, and
        # step 1's in-place v write must wait for this DMA's read of v
        nc.scalar.dma_start(vstore[0], v[:])

        # ---- forward ----
        # Per step: scores chunks produced by DVE ("D") or Pool ("P") adds;
        # all grouped max-reduces on DVE. jr-group chunk plan, F-half
        # (jr 32:64, feeds the PE swap) first; trailing Pool chunks shrink so
        # the DVE tail isn't serialized behind a big Pool chunk.
        from concourse.tile_rust import add_dep_helper
        # All DVE-added groups live in the F-half so the D add+reduce are one
        # instruction each and mhF completes early (PE swap off critical path).
        PLAN = [("D", 32, 32 + 2 * DA),
                ("P", 32 + 2 * DA, 64),
                ("P", 0, 13), ("P", 13, 26), ("P", 26, 32)]
        for s in range(1, n_steps):
            e_s = t_feats[:, s * H:(s + 1) * H]
            vb = v[:, None, :]
            mhF = mpool.tile([P2, G], F32, tag="mhF")
            mhO = mpool.tile([P2, G], F32, tag="mhO")

            # adds first (DVE chunks immediately; Pool chunks in queue order)
            chunks = []
            prevp = None
            for ci, (eng, j0, j1) in enumerate(PLAN):
                sc = spool.tile([P2, (j1 - j0) * H], F32, tag=f"sc{ci}")
                scv = sc[:].rearrange("p (j i) -> p j i", i=H)
                args = (scv, r3(t_transbiR[:])[:, j0:j1, :],
                        vb.to_broadcast([P2, j1 - j0, H]))
                if eng == "D":
                    nc.vector.tensor_add(*args)
                else:
                    p = nc.gpsimd.tensor_add(*args)
                    if prevp is not None:
                        add_dep_helper(p.ins, prevp.ins, sync=False,
                                       reason="pool chunk order")
                    prevp = p
                chunks.append((scv, j0, j1))

            # grouped reduces on DVE, in plan order
            prevr = None
            for ci, (scv, j0, j1) in enumerate(chunks):
                if j0 >= G:
                    dst = mhF[:, j0 - G:j1 - G]
                else:
                    dst = mhO[:, j0:j1]
                r = nc.vector.tensor_reduce(dst, scv, axis=AX.X, op=A.max)
                if prevr is not None:
                    add_dep_helper(r.ins, prevr.ins, sync=False,
                                   reason="reduce order on DVE")
                prevr = r
                if j1 == 2 * G:  # mhF complete -> cross-half swap on PE
                    msw = pspool.tile([P2, G], F32, tag="msw")
                    nc.tensor.matmul(msw[:], t_swap[:], mhF[:],
                                     start=True, stop=True)

            if s < S // 2:
                # lengths are >= S//2, so mask == 1 on every lane here: the
                # blend is an unconditional write and the e-add can target
                # the state tile directly (same WAR pattern as copy_pred)
                best = mpool.tile([P2, G], F32, tag="beste")
                nc.vector.tensor_tensor(best[:], mhO[:], msw[:], op=A.max)
                nc.vector.tensor_add(v[:], best[:], e_s)
            else:
                beste = mpool.tile([P2, G], F32, tag="beste")
                nc.vector.tensor_tensor(beste[:], mhO[:], msw[:], op=A.max)
                nc.vector.tensor_add(beste[:], beste[:], e_s)
                nc.vector.copy_predicated(v[:],
                                          t_m2i[:, s:s + 1].to_broadcast([P2, H]),
                                          beste[:])
            if s < n_steps - 1:  # vstore[n_steps-1] is never read back
                nc.sync.dma_start(vstore[s], v[:])

        # ---- epilogue: last_path ----
        vnat = statics.tile([BL, T], F32)
        nc.vector.tensor_copy(vnat[:, 0:H], v[0:BL, :])
        nc.sync.dma_start(vnat[:, H:T], v[BL:P2, :])

        fv = statics.tile([BL, T], F32)
        nc.vector.tensor_add(fv[:], vnat[:], t_endb[:])
        fv8 = statics.tile([BL, 8], F32)
        nc.vector.max(out=fv8[:], in_=fv[:])
        fvi = statics.tile([BL, 8], U32)
        nc.vector.max_index(fvi[:], fv8[:], fv[:])
        nc.vector.tensor_copy(t_dec[:, n_steps - 1:n_steps], fvi[:, 0:1])
        nc.vector.tensor_scalar(t_addend[:], t_onehL[:],
                                t_dec[:, n_steps - 1:n_steps], None, op0=A.mult)

        # ---- backtrack: batched v/e prefetch, per-step argmax recompute ----
        # chunk c covers steps [c*W, c*W+W); processed descending
        nch = (n_steps - 1 + W - 1) // W

        vst4 = vstore.rearrange("s (h b) i -> s h b i", h=2)

        def prefetch(c):
            c0 = c * W
            cw = min(W, (n_steps - 1) - c0)  # steps c0..c0+cw-1
            if cw <= 0:
                return None, None, 0
            vt = ringpool.tile([BL, W * T], F32, tag="vt")
            src = vst4[c0:c0 + cw].rearrange("s h b i -> b s h i")
            nc.scalar.dma_start(
                vt[:].rearrange("b (s h i) -> b s h i", h=2, i=H)[:, 0:cw], src)
            et = ringpool.tile([BL, W * T], F32, tag="et")
            esrc = featsn[c0 + 1:c0 + cw + 1].rearrange("s b t -> b s t")
            nc.scalar.dma_start(
                et[:].rearrange("b (s t) -> b s t", t=T)[:, 0:cw, :], esrc)
            return vt, et, cw

        bufs = {}
        for c in range(nch - 1, max(nch - 3, -1), -1):
            bufs[c] = prefetch(c)

        for i in range(n_steps - 2, -1, -1):
            c = i // W
            w = i - c * W
            if c - 2 >= 0 and c - 2 not in bufs:
                bufs[c - 2] = prefetch(c - 2)
            vt_t, et_t, cw = bufs[c]
            vt = vt_t[:, w * T:(w + 1) * T]
            et = et_t[:, w * T:(w + 1) * T]

            lp_ap = t_dec[:, i + 1:i + 2]
            onehot = btpool.tile([BL, T], F32, tag="onehot")
            nc.vector.tensor_scalar(onehot[:], t_iota[:], lp_ap, None,
                                    op0=A.is_equal)

            prod = btpool.tile([BL, T], F32, tag="prod")
            nc.vector.tensor_mul(prod[:], et, onehot[:])
            elp = btpool.tile([BL, 1], F32, tag="elp")
            nc.vector.tensor_reduce(elp[:], prod[:], axis=AX.X, op=A.add)

            p_ohT = pspool.tile([T, BL], F32, tag="p_ohT")
            nc.tensor.transpose(p_ohT[:], onehot[:], t_ident[:])
            ohT = btpool.tile([T, BL], F32, tag="ohT")
            nc.vector.tensor_copy(ohT[:], p_ohT[:])
            p_tlp = pspool.tile([BL, T], F32, tag="p_tlp")
            # psum = I @ vt first (no ohT dependency - fires early), then
            # accumulate trans[:, lp].T: fl(v + T_lp), exact (2 addends)
            nc.tensor.matmul(p_tlp[:], t_ident[:], vt, start=True, stop=False)
            nc.tensor.matmul(p_tlp[:], ohT[:], t_transT[:], start=False, stop=True)

            cand3 = btpool.tile([BL, T], F32, tag="cand3")
            nc.vector.tensor_scalar(cand3[:], p_tlp[:], elp[:, 0:1], None,
                                    op0=A.add)

            c8 = btpool.tile([BL, 8], F32, tag="c8")
            nc.vector.max(out=c8[:], in_=cand3[:])
            ci = btpool.tile([BL, 8], U32, tag="ci")
            nc.vector.max_index(ci[:], c8[:], cand3[:])

            nc.vector.tensor_scalar(t_dec[:, i:i + 1], ci[:, 0:1],
                                    t_m[:, i + 1:i + 2], t_addend[:, i:i + 1],
                                    op0=A.mult, op1=A.add)

        nc.sync.dma_start(dec_out, t_dec[:])

    nc.compile()
    return nc


def host_prep(feats, mask, start_transitions, end_transitions, transitions,
              n_steps=S):
    feats = np.asarray(feats, dtype=np.float32)
    mask = np.asarray(mask, dtype=np.float32)
    start = np.asarray(start_transitions, dtype=np.float32)
    end = np.asarray(end_transitions, dtype=np.float32)
    trans = np.asarray(transitions, dtype=np.float32)

    # transbiR[(ihi*BL+b), jr*H+i32] = trans[ihi*H+i32, jmap(ihi, jr)]
    # jmap(0, jr) = jr; jmap(1, jr) = (jr + H) % T  (own j's first per group)
    transbiR = np.empty((P2, T * H), dtype=np.float32)
    blk0 = np.ascontiguousarray(trans[0:H, :].T)           # [T(j), H(i32)]
    transbiR[0:BL, :] = np.tile(blk0.reshape(1, T * H), (BL, 1))
    blk1 = np.ascontiguousarray(trans[H:T, :].T)           # [T(j), H(i32)]
    blk1r = np.concatenate([blk1[H:], blk1[:H]], axis=0)   # j = (jr+H)%T
    transbiR[BL:P2, :] = np.tile(blk1r.reshape(1, T * H), (BL, 1))

    transT = np.ascontiguousarray(trans.T)
    startsp = np.empty((P2, H), dtype=np.float32)
    for ihi in range(2):
        startsp[ihi * BL:(ihi + 1) * BL, :] = np.tile(
            start[ihi * H:(ihi + 1) * H].reshape(1, H), (BL, 1))
    swapd = np.roll(np.eye(P2, dtype=np.float32), BL, axis=0)
    endb = np.tile(end.reshape(1, T), (BL, 1))
    iotad = np.tile(np.arange(T, dtype=np.float32).reshape(1, T), (BL, 1))
    identd = np.eye(T, dtype=np.float32)

    lengths = mask.sum(axis=1).astype(np.int64)

    in_maps = []
    for c in range(NCORES):
        b0 = c * BL
        fc = feats[:n_steps, b0:b0 + BL, :]                      # [S, BL, T]
        featsp = np.ascontiguousarray(
            fc.reshape(n_steps, BL, 2, H).transpose(2, 1, 0, 3)
        ).reshape(P2, n_steps * H)
        msk = np.ascontiguousarray(mask[b0:b0 + BL, :n_steps])
        msk2 = np.concatenate([msk, msk], axis=0)
        onehL = (np.arange(n_steps)[None, :] == (lengths[b0:b0 + BL, None] - 1))
        in_maps.append(dict(
            featsp=featsp,
            featsn=np.ascontiguousarray(fc),
            transbiR=transbiR, transTd=transT, startsp=startsp, swapd=swapd,
            m2i_all=msk2.astype(np.int32),
            m_all=msk, onehL=onehL.astype(np.float32),
            endb=endb, iotad=iotad, identd=identd,
        ))
    return in_maps


def kernel(feats, mask, start_transitions, end_transitions, transitions):
    if "nc" not in _cached:
        _cached["nc"] = build_program(S)
    nc = _cached["nc"]
    in_maps = host_prep(feats, mask, start_transitions, end_transitions,
                        transitions, S)
    res = run_bass_kernel_spmd(nc, in_maps, list(range(NCORES)))
    out = np.empty((B, S), dtype=np.int32)
    for c in range(NCORES):
        out[c * BL:(c + 1) * BL, :] = np.rint(
            res.results[c]["dec_out"]).astype(np.int32)
    return out
